# revision 3
# baseline (speedup 1.0000x reference)
"""Trainium2 Bass kernel v2 for nn_DecoderLayer_50534585205086.

Sharding: 8 cores = 4 batches x 2 sequence halves; T=2048 tokens/core.
All GEMM operands bf16 (PSUM accumulation f32). Key structure vs v1:
 - dwconv+avgpool fused into a 6-tap stride-4 FIR applied to the bf16
   projection outputs (taps precomputed on host).
 - attention output, wc projection and LN1 all computed at pooled
   resolution (upsampled attention is piecewise-constant over KER=4
   tokens); upsample happens in the x1 = xemb + up(LN1) add via strided
   views.
 - xemb and x1 stay resident in SBUF (no DRAM round-trips).
 - LN2 + residual computed token-major after PE transposes of ffw and
   x1, with fused accum_out sums for the stats; y written token-major.
"""

import numpy as np
from contextlib import ExitStack

import concourse.bass as bass
import concourse.tile as tile
from concourse import bacc, mybir
from concourse.bass import ts
from concourse.bass_utils import run_bass_kernel_spmd
from concourse.masks import make_identity

F32 = mybir.dt.float32
BF16 = mybir.dt.bfloat16
AL = mybir.AluOpType
AF = mybir.ActivationFunctionType

N_CORES = 8
B, S_FULL, DM, H, DD, DF = 4, 4096, 1024, 16, 64, 4096
KER, KW = 4, 3
NORM = float(DD) ** -0.25
EPS = 1e-6
CT = DM // 128  # 8 channel tiles
FT = DF // 128  # 32 ffn tiles
HALO = 8
NTAP = KER + KW - 1  # 6 combined FIR taps

# packed constant-vector column offsets: name -> (offset, width)
_COFF = {}
_off = 0
for _nm, _w in [("firq", CT * NTAP), ("firk", CT * NTAP), ("firv", CT * NTAP),
                ("dbq", CT), ("dbk", CT), ("dbv", CT),
                ("fxq", CT), ("fxk", CT), ("fxv", CT),
                ("bqn", CT), ("bkn", CT), ("bv", CT), ("bc", CT),
                ("g1", CT), ("be1", CT),
                ("b1", FT), ("b2", CT), ("bup", 1), ("hmask", HALO)]:
    _COFF[_nm] = (_off, _w)
    _off += _w
NCONST = _off


def build_program(S=S_FULL, mock_collective=False, debug=False):
    T = S // 2           # tokens per core
    TH = T + HALO
    L = S // KER         # pooled length per batch
    LLOC = L // 2        # pooled positions owned per core
    MT = L // 128        # m tiles (keys)
    TB = T // 128        # output token blocks

    nc = bacc.Bacc("TRN2", target_bir_lowering=False, debug=False,
                   num_devices=N_CORES)

    def din(name, shape, dt=F32):
        return nc.dram_tensor(name, list(shape), dt, kind="ExternalInput").ap()

    xemb_ap = din("xemb", [DM, TH], BF16)  # host: (x_enc+x_pos), bf16,
    # residue-major token order: col r*(TH//4)+q holds token 4q+r-HALO
    QB = TH // 4  # 514: cols per residue block
    wq_ap = din("wq", [CT, 128, DM], BF16)
    wk_ap = din("wk", [CT, 128, DM], BF16)
    wv_ap = din("wv", [CT, 128, DM], BF16)
    wc_ap = din("wc", [CT, 128, DM], BF16)
    w1_ap = din("w1", [FT, 128, DM], BF16)
    w2_ap = din("w2", [FT, 128, DM], BF16)
    wup_ap = din("wup", [DD, DD], BF16)
    cvec_ap = din("cvec", [128, NCONST])
    mask_ap = din("mask", [L, LLOC], BF16)
    g2bc_ap = din("g2bc", [128, DM], BF16)
    b2bc_ap = din("b2bc", [128, DM], BF16)
    be2bc_ap = din("be2bc", [128, DM], BF16)

    y_ap = nc.dram_tensor("y", [T, DM], F32, kind="ExternalOutput").ap()
    dbg = {}
    if debug:
        for nm, sh in (("dqp", [DM, LLOC]), ("dop", [DM, LLOC]),
                       ("dx1", [DM, T])):
            dbg[nm] = nc.dram_tensor(nm, sh, BF16,
                                     kind="ExternalOutput").ap()
    dbg = {}
    if debug:
        for nm, sh in (("dqp", [DM, LLOC]), ("dop", [DM, LLOC]),
                       ("dx1", [DM, T])):
            dbg[nm] = nc.dram_tensor(nm, sh, BF16,
                                     kind="ExternalOutput").ap()

    with tile.TileContext(nc) as tc, ExitStack() as ctx:
        const = ctx.enter_context(tc.tile_pool(name="const", bufs=1))
        dram = ctx.enter_context(tc.tile_pool(name="dram", bufs=1, space="DRAM"))

        kv_local = dram.tile([2, H, DD, LLOC], BF16, tag="kv_local")
        kv_all = dram.tile([2, 2, H, DD, LLOC], BF16, tag="kv_all")

        # ---- constants ----
        ident = const.tile([128, 128], F32, tag="ident")
        make_identity(nc, ident)
        ident_b = const.tile([128, 128], BF16, tag="ident_b")
        nc.vector.tensor_copy(ident_b, ident)
        ones_row = const.tile([1, 128], BF16, tag="ones_row")
        nc.vector.memset(ones_row, 1.0)
        ones_col = const.tile([128, 1], BF16, tag="ones_col")
        nc.vector.memset(ones_col, 1.0)
        eps_t = const.tile([1, 1], F32, tag="eps_t")
        nc.vector.memset(eps_t, EPS)
        eps_c = const.tile([128, 1], F32, tag="eps_c")
        nc.vector.memset(eps_c, EPS)
        wup_sb = const.tile([DD, DD], BF16, tag="wup_sb")
        nc.sync.dma_start(out=wup_sb, in_=wup_ap)
        g2bc = const.tile([128, DM], BF16, tag="g2bc")
        nc.sync.dma_start(out=g2bc, in_=g2bc_ap)
        be2bc = const.tile([128, DM], BF16, tag="be2bc")
        nc.sync.dma_start(out=be2bc, in_=be2bc_ap)
        b2bc = const.tile([128, DM], BF16, tag="b2bc")
        nc.sync.dma_start(out=b2bc, in_=b2bc_ap)

        cvec_t = const.tile([128, NCONST], F32, tag="cvec_t")
        nc.sync.dma_start(out=cvec_t, in_=cvec_ap)


        def cslice(nm):
            off, w = _COFF[nm]
            return cvec_t[:, off:off + w]

        fir = {"q": cslice("firq"), "k": cslice("firk"), "v": cslice("firv")}
        dbs = {"q": cslice("dbq"), "k": cslice("dbk"), "v": cslice("dbv")}
        fxs = {"q": cslice("fxq"), "k": cslice("fxk"), "v": cslice("fxv")}
        bias_qkv = {"q": cslice("bqn"), "k": cslice("bkn"), "v": cslice("bv")}
        bc_t = cslice("bc")
        g1_t = cslice("g1")
        be1_t = cslice("be1")
        b1_t = cslice("b1")
        b2_t = cslice("b2")
        bup_t = cslice("bup")
        hm_t = cslice("hmask")

        # right-side pools release LIFO: create in reverse release order
        # (release: qp after D; op, xemb after E; x1 at end)
        x1_ctx = ExitStack()
        x1_pool = x1_ctx.enter_context(
            tc.tile_pool(name="x1_pool", bufs=1, side="right"))
        x1_tiles = [x1_pool.tile([128, T], BF16, tag=f"x1_{i}",
                                 name=f"x1_{i}") for i in range(CT)]
        xemb_ctx = ExitStack()
        xemb_pool = xemb_ctx.enter_context(
            tc.tile_pool(name="xemb_pool", bufs=1, side="right"))
        xemb_tiles = [xemb_pool.tile([128, TH], BF16, tag=f"xemb{i}",
                                     name=f"xemb{i}") for i in range(CT)]
        op_ctx = ExitStack()
        op_pool = op_ctx.enter_context(
            tc.tile_pool(name="op_pool", bufs=1, side="right"))
        opool_tiles = [op_pool.tile([128, LLOC], BF16, tag=f"opool{i}",
                                    name=f"opool{i}") for i in range(CT)]
        mask_ctx = ExitStack()
        mask_pool = mask_ctx.enter_context(
            tc.tile_pool(name="mask_pool", bufs=1, side="right"))
        mask_tiles = []
        for _mt in range(MT):
            _m = mask_pool.tile([128, LLOC], BF16, tag=f"mask{_mt}",
                                name=f"mask{_mt}")
            nc.sync.dma_start(out=_m, in_=mask_ap[ts(_mt, 128), :])
            mask_tiles.append(_m)
        qp_ctx = ExitStack()
        qp_pool = qp_ctx.enter_context(
            tc.tile_pool(name="qp_pool", bufs=1, side="right"))
        qp_tiles = [qp_pool.tile([128, LLOC], BF16, tag=f"qp{i}",
                                 name=f"qp{i}") for i in range(CT)]

        proj_chunks = []
        c0 = 0
        while c0 < TH:
            proj_chunks.append((c0, min(512, TH - c0)))
            c0 += 512

        # ================= Stage A+B: embed, QKV proj, FIR pool =============
        with tc.tile_pool(name="sAB", bufs=1) as sab, \
             tc.tile_pool(name="psAB", bufs=1, space="PSUM") as psab:
            xemb_r = xemb_ap.rearrange("(c p) t -> c p t", p=128)
            for ci in range(CT):
                nc.sync.dma_start(out=xemb_tiles[ci], in_=xemb_r[ci])

            def emit_proj(kind, w_ap):
                bias_t = bias_qkv[kind]
                for co in range(CT):
                    wt = sab.tile([128, DM], BF16, tag="wblk", bufs=3,
                                  name=f"w{kind}{co}")
                    nc.scalar.dma_start(out=wt, in_=w_ap[co])
                    pre = sab.tile([128, TH], BF16, tag="pre", bufs=3,
                                   name=f"pre{kind}{co}")
                    for (c0, cw) in proj_chunks:
                        ps = psab.tile([128, 512], F32, tag="qkv", bufs=3,
                                       name=f"ps{kind}{co}_{c0}")
                        for ci in range(CT):
                            nc.tensor.matmul(
                                ps[:, :cw], wt[:, ts(ci, 128)],
                                xemb_tiles[ci][:, c0:c0 + cw],
                                start=(ci == 0), stop=(ci == CT - 1))
                        nc.scalar.activation(pre[:, c0:c0 + cw], ps[:, :cw],
                                             AF.Identity,
                                             bias=bias_t[:, co:co + 1])
                    # zero the bias-injected halo cols (tokens -8..-1 live at
                    # q 0..1 of each residue block) on first-half cores
                    hv = pre.rearrange("p (r q) -> p r q", q=QB)[:, :, 0:2]
                    nc.vector.tensor_scalar(hv, hv, hm_t[:, 0:1], None,
                                            op0=AL.mult)

                    # fused dwconv+pool: out[l] = sum_j g_j * pre[tok 4l-5+j];
                    # token t at col ((t+HALO)%4)*QB + (t+HALO)//4 -> tap j
                    # reads the packed slice [rj*QB+oj : +LLOC],
                    # rj=(j+3)%4, oj=(j+3)//4
                    ft = fir[kind]

                    def dec(j):
                        rj, oj = (j + 3) % 4, (j + 3) // 4
                        return pre[:, rj * QB + oj: rj * QB + oj + LLOC]

                    fo = co * NTAP
                    # 6 tap products on the Activation engine (per-channel
                    # scale), add-tree on DVE/Pool: stt has no DVE fast mode
                    prods = []
                    for j in range(NTAP):
                        pj = sab.tile([128, LLOC], BF16, tag=f"fp{j}",
                                      bufs=2, name=f"fp{j}{kind}{co}")
                        if j in (1, 4):
                            nc.vector.tensor_scalar_mul(
                                pj, dec(j), ft[:, fo + j:fo + j + 1])
                        else:
                            nc.scalar.activation(
                                pj, dec(j), AF.Identity,
                                bias=(dbs[kind][:, co:co + 1] if j == 0
                                      else 0.0),
                                scale=ft[:, fo + j:fo + j + 1])
                        prods.append(pj)
                    s1 = sab.tile([128, LLOC], BF16, tag="fs1", bufs=2,
                                  name=f"fs1{kind}{co}")
                    nc.vector.tensor_add(s1, prods[0], prods[1])
                    s2 = sab.tile([128, LLOC], BF16, tag="fs2", bufs=2,
                                  name=f"fs2{kind}{co}")
                    nc.gpsimd.tensor_add(s2, prods[2], prods[3])
                    s3 = sab.tile([128, LLOC], BF16, tag="fs3", bufs=2,
                                  name=f"fs3{kind}{co}")
                    nc.vector.tensor_add(s3, prods[4], prods[5])
                    s4 = sab.tile([128, LLOC], BF16, tag="fs4", bufs=2,
                                  name=f"fs4{kind}{co}")
                    nc.gpsimd.tensor_add(s4, s1, s2)
                    if kind == "q":
                        nc.vector.tensor_add(qp_tiles[co], s3, s4)
                        # first-pooled-position bias fix (hf=0 cores)
                        nc.vector.tensor_add(qp_tiles[co][:, 0:1],
                                             qp_tiles[co][:, 0:1],
                                             fxs[kind][:, co:co + 1])
                    else:
                        kvp = sab.tile([128, LLOC], BF16, tag="kvp", bufs=3,
                                       name=f"kvp{kind}{co}")
                        nc.vector.tensor_add(kvp, s3, s4)
                        nc.vector.tensor_add(kvp[:, 0:1], kvp[:, 0:1],
                                             fxs[kind][:, co:co + 1])
                        kvi = 0 if kind == "k" else 1
                        nc.gpsimd.dma_start(
                            out=kv_local[kvi, 2 * co:2 * co + 2].rearrange(
                                "h d m -> (h d) m"),
                            in_=kvp)

            emit_proj("k", wk_ap)
            emit_proj("v", wv_ap)
            # ============= Stage C: AllGather pooled K/V (overlaps q) =======
            if mock_collective:
                nc.sync.dma_start(out=kv_all[0], in_=kv_local)
                nc.sync.dma_start(out=kv_all[1], in_=kv_local)
            else:
                nc.gpsimd.collective_compute(
                    "AllGather", AL.bypass,
                    replica_groups=[[0, 1], [2, 3], [4, 5], [6, 7]],
                    ins=[kv_local.opt()], outs=[kv_all.opt()])
            emit_proj("q", wq_ap)

        # ============ Stage D: pooled causal attention + wup ================
        with tc.tile_pool(name="sD", bufs=1) as sd, \
             tc.tile_pool(name="psD", bufs=1, space="PSUM") as psd:
            CW = DD + 1  # vpt chunk: DD value cols + 1 ones col

            def emit_front(h):
                """kv loads, transposes, logits+exp+mask for head h."""
                hp, j = h // 2, h % 2
                if j == 0:
                    kp2 = sd.tile([128, L], BF16, tag="kp2", bufs=2,
                                  name=f"kp2_{hp}")
                    nc.sync.dma_start(
                        out=kp2,
                        in_=kv_all[:, 0, 2 * hp:2 * hp + 2].rearrange(
                            "g h d m -> (h d) g m"))
                    emit_front.kp2 = kp2
                kp2 = emit_front.kp2
                vp_h = sd.tile([DD, L], BF16, tag="vph", bufs=2,
                               name=f"vp{h}")
                nc.sync.dma_start(
                    out=vp_h,
                    in_=kv_all[:, 1, h].rearrange("g d m -> d g m"))
                qp_h = qp_tiles[hp][j * DD:(j + 1) * DD, :]
                kp_h = kp2[j * DD:(j + 1) * DD, :]

                ps_trh = psd.tile([128, MT * DD], BF16, tag="trh",
                                  bufs=2, name=f"trh{h}")
                for mt in range(MT):
                    nc.tensor.transpose(
                        ps_trh[:, mt * DD:(mt + 1) * DD],
                        vp_h[:, ts(mt, 128)], ident_b[0:DD, 0:DD])
                vpt = sd.tile([128, MT, CW], BF16, tag="vpt",
                              bufs=2, name=f"vpt{h}")
                nc.vector.memset(vpt[:, :, DD:CW], 1.0)
                nc.vector.tensor_copy(
                    vpt[:, :, 0:DD],
                    ps_trh.rearrange("p (m d) -> p m d", d=DD))

                wexpms = []
                for mt in range(MT):
                    ps_lg = psd.tile([128, LLOC], F32, tag="lg", bufs=2,
                                     name=f"lg{h}_{mt}")
                    nc.tensor.matmul(ps_lg, kp_h[:, ts(mt, 128)], qp_h,
                                     start=True, stop=True,
                                     tile_position=(j * DD, 0))
                    wexp = sd.tile([128, LLOC], BF16, tag="wexp",
                                   bufs=4, name=f"wexp{h}_{mt}")
                    nc.scalar.activation(wexp, ps_lg, AF.Exp)
                    wexpm = sd.tile([128, LLOC], BF16, tag="wexpm",
                                    bufs=2 * MT + 2, name=f"wexpm{h}_{mt}")
                    eng = nc.gpsimd if mt % 8 < 3 else nc.vector
                    eng.tensor_mul(wexpm, wexp, mask_tiles[mt])
                    wexpms.append(wexpm)
                return (h, vpt, wexpms)

            def emit_back(st):
                """AV accumulation + wup + normalize for a prepared head."""
                h, vpt, wexpms = st
                hp, j = h // 2, h % 2
                ps_av = psd.tile([DD + 1, LLOC], F32, tag="av", bufs=2,
                                 name=f"av{h}")
                for mt in range(MT):
                    nc.tensor.matmul(ps_av, vpt[:, mt, :], wexpms[mt],
                                     start=(mt == 0), stop=(mt == MT - 1))
                o_sb = sd.tile([DD, LLOC], BF16, tag="osb", bufs=2,
                               name=f"osb{h}")
                nc.vector.tensor_copy(o_sb, ps_av[0:DD, :])
                rec = sd.tile([1, LLOC], BF16, tag="rec", bufs=2,
                              name=f"rec{h}")
                with nc.allow_low_precision(reason="softmax denom recip"):
                    nc.vector.reciprocal(rec, ps_av[DD:DD + 1, :])
                ps_ob = psd.tile([128, LLOC], F32, tag="ob", bufs=2,
                                 name=f"ob{h}")
                nc.tensor.matmul(ps_ob[0:DD, :], wup_sb, o_sb,
                                 start=True, stop=True)
                nc.tensor.matmul(ps_ob[DD:2 * DD, :],
                                 ones_row[0:1, 0:DD], rec,
                                 start=True, stop=True,
                                 tile_position=(0, DD))
                bc_sb = sd.tile([DD, LLOC], F32, tag="bcs", bufs=2,
                                name=f"bcs{h}")
                nc.vector.tensor_copy(bc_sb, ps_ob[DD:2 * DD, :])
                own = sd.tile([DD, LLOC], F32, tag="own", bufs=2,
                              name=f"own{h}")
                nc.vector.tensor_mul(own, ps_ob[0:DD, :], bc_sb)
                nc.scalar.activation(
                    opool_tiles[hp][j * DD:(j + 1) * DD, :], own,
                    AF.Identity, bias=bup_t[0:DD, 0:1])

            pending = []
            for h in range(H):
                pending.append(emit_front(h))
                if len(pending) > 1:
                    emit_back(pending.pop(0))
            for st in pending:
                emit_back(st)

        if debug:
            for i in range(CT):
                nc.sync.dma_start(out=dbg["dqp"][ts(i, 128), :],
                                  in_=qp_tiles[i])
                nc.sync.dma_start(out=dbg["dop"][ts(i, 128), :],
                                  in_=opool_tiles[i])
        if debug:
            for i in range(CT):
                nc.sync.dma_start(out=dbg["dqp"][ts(i, 128), :],
                                  in_=qp_tiles[i])
                nc.sync.dma_start(out=dbg["dop"][ts(i, 128), :],
                                  in_=opool_tiles[i])
        qp_ctx.close()
        mask_ctx.close()

        # ============ Stage E: pooled wc proj + LN1 + x1 assembly ===========
        # split into 2 pooled-position halves so FFN1(mc=0) can start while
        # the second half is still in flight
        with tc.tile_pool(name="sE", bufs=1) as se, \
             tc.tile_pool(name="psE", bufs=1, space="PSUM") as pse:
            wc_tiles = []
            for co in range(CT):
                wct = se.tile([128, DM], BF16, tag=f"wcb{co}", bufs=1,
                              name=f"wcb{co}")
                nc.scalar.dma_start(out=wct, in_=wc_ap[co])
                wc_tiles.append(wct)
            EH = LLOC // 2
            for eh in range(2):
                e0 = eh * EH
                ps_s1 = pse.tile([1, EH], F32, tag="s1", bufs=1,
                                 name=f"s1_{eh}")
                ps_s2 = pse.tile([1, EH], F32, tag="s2", bufs=1,
                                 name=f"s2_{eh}")
                a_tiles = []
                for co in range(CT):
                    ps_wc = pse.tile([128, EH], F32, tag="wc", bufs=2,
                                     name=f"pswc{co}_{eh}")
                    for ci in range(CT):
                        nc.tensor.matmul(ps_wc, wc_tiles[co][:, ts(ci, 128)],
                                         opool_tiles[ci][:, e0:e0 + EH],
                                         start=(ci == 0), stop=(ci == CT - 1))
                    a_sb = se.tile([128, EH], BF16, tag=f"asb{co}", bufs=2,
                                   name=f"asb{co}_{eh}")
                    nc.scalar.activation(a_sb, ps_wc, AF.Identity,
                                         bias=bc_t[:, co:co + 1])
                    a2 = se.tile([128, EH], BF16, tag="a2", bufs=2,
                                 name=f"a2_{co}_{eh}")
                    nc.vector.tensor_mul(a2, a_sb, a_sb)
                    nc.tensor.matmul(ps_s1, ones_col, a_sb,
                                     start=(co == 0), stop=(co == CT - 1))
                    nc.tensor.matmul(ps_s2, ones_col, a2,
                                     start=(co == 0), stop=(co == CT - 1))
                    a_tiles.append(a_sb)

                mean_b = se.tile([1, EH], BF16, tag="meanb", bufs=2,
                                 name=f"meanb{eh}")
                nc.vector.tensor_scalar_mul(mean_b, ps_s1, 1.0 / DM)
                e2 = se.tile([1, EH], F32, tag="e2", bufs=2, name=f"e2_{eh}")
                nc.vector.tensor_scalar_mul(e2, ps_s2, 1.0 / DM)
                m2 = se.tile([1, EH], F32, tag="m2", bufs=2, name=f"m2_{eh}")
                nc.vector.tensor_mul(m2, mean_b, mean_b)
                var = se.tile([1, EH], F32, tag="var", bufs=2,
                              name=f"var{eh}")
                nc.vector.tensor_sub(var, e2, m2)
                sd_t = se.tile([1, EH], F32, tag="sd", bufs=2,
                               name=f"sd{eh}")
                nc.scalar.activation(sd_t, var, AF.Sqrt, bias=eps_t[0:1, 0:1])
                rstd_b = se.tile([1, EH], BF16, tag="rstdb", bufs=2,
                                 name=f"rstdb{eh}")
                with nc.allow_low_precision(reason="bf16 rstd"):
                    nc.vector.reciprocal(rstd_b, sd_t)

                ps_mb = pse.tile([128, EH], F32, tag="mb", bufs=2,
                                 name=f"mb{eh}")
                nc.tensor.matmul(ps_mb, ones_row, mean_b,
                                 start=True, stop=True)
                ps_rb = pse.tile([128, EH], F32, tag="rb", bufs=2,
                                 name=f"rb{eh}")
                nc.tensor.matmul(ps_rb, ones_row, rstd_b,
                                 start=True, stop=True)
                mb_sb = se.tile([128, EH], BF16, tag="mbs", bufs=2,
                                name=f"mbs{eh}")
                nc.vector.tensor_copy(mb_sb, ps_mb)
                rb_sb = se.tile([128, EH], BF16, tag="rbs", bufs=2,
                                name=f"rbs{eh}")
                nc.vector.tensor_copy(rb_sb, ps_rb)

                for co in range(CT):
                    v1 = se.tile([128, EH], BF16, tag="lnv", bufs=2,
                                 name=f"lnv{co}_{eh}")
                    nc.vector.tensor_sub(v1, a_tiles[co], mb_sb)
                    v2 = se.tile([128, EH], BF16, tag="lnu", bufs=2,
                                 name=f"lnu{co}_{eh}")
                    nc.vector.tensor_mul(v2, v1, rb_sb)
                    v3 = se.tile([128, EH], BF16, tag="lnw", bufs=2,
                                 name=f"lnw{co}_{eh}")
                    nc.vector.tensor_scalar(v3, v2, g1_t[:, co:co + 1],
                                            be1_t[:, co:co + 1],
                                            op0=AL.mult, op1=AL.add)
                    # x1 = xemb + upsample4(v3); both residue-major so all
                    # slices are packed: x1 col r*(T//4)+l = token 4l+r,
                    # matching xemb col r*QB + (l+2)
                    for r in range(KER):
                        x1s = x1_tiles[co][:, r * (T // 4) + e0:
                                           r * (T // 4) + e0 + EH]
                        xes = xemb_tiles[co][:, r * QB + 2 + e0:
                                             r * QB + 2 + e0 + EH]
                        eng = nc.vector if r < 2 else nc.gpsimd
                        eng.tensor_add(x1s, v3, xes)

        if debug:
            for i in range(CT):
                nc.sync.dma_start(out=dbg["dx1"][ts(i, 128), :],
                                  in_=x1_tiles[i])
        if debug:
            for i in range(CT):
                nc.sync.dma_start(out=dbg["dx1"][ts(i, 128), :],
                                  in_=x1_tiles[i])
        op_ctx.close()
        xemb_ctx.close()

        # ======== Stage F: FFN + token-major FFN2/LN2 + residual ============
        with tc.tile_pool(name="sF", bufs=1) as sf, \
             tc.tile_pool(name="psF", bufs=1, space="PSUM") as psf:
            w2_tiles = []

            def load_w2():
                # resident w2, natural layout (moving operand of FFN2);
                # emitted after mc=0's w1 stream so it doesn't head-of-line
                # block FFN1's weights on the scalar DMA queue
                for f in range(FT):
                    w2t = sf.tile([128, DM], BF16, tag=f"w2r{f}",
                                  name=f"w2r{f}")
                    nc.scalar.dma_start(out=w2t, in_=w2_ap[f])
                    w2_tiles.append(w2t)

            for mc in range(2):
                mc0 = mc * (T // 2)
                hb_tiles = []
                for f in range(FT):
                    w1t = sf.tile([128, DM], BF16, tag="w1b", bufs=2,
                                  name=f"w1t{f}_{mc}")
                    nc.scalar.dma_start(out=w1t, in_=w1_ap[f])
                    hb = sf.tile([128, T // 2], BF16, tag=f"hb{f}",
                                 name=f"hb{f}_{mc}")
                    for h2 in range(2):
                        q0 = mc0 + h2 * 512
                        ps_h = psf.tile([128, 512], F32, tag="fps", bufs=2,
                                        name=f"psh{f}_{mc}_{h2}")
                        for ci in range(CT):
                            nc.tensor.matmul(ps_h, w1t[:, ts(ci, 128)],
                                             x1_tiles[ci][:, q0:q0 + 512],
                                             start=(ci == 0),
                                             stop=(ci == CT - 1))
                        hr = sf.tile([128, 512], BF16, tag="hr", bufs=2,
                                     name=f"hr{f}_{mc}_{h2}")
                        nc.scalar.activation(hr, ps_h, AF.Relu,
                                             bias=b1_t[:, f:f + 1])
                        nc.gpsimd.tensor_mul(hb[:, ts(h2, 512)], hr, hr)
                    hb_tiles.append(hb)
                if not w2_tiles:
                    load_w2()

                # FFN2 token-major + fused LN2 + residual per token block
                for tb in range(TB // 2):
                    t0 = mc0 + tb * 128
                    tloc = tb * 128
                    ps_y0 = psf.tile([128, 512], F32, tag="yps0", bufs=2,
                                     name=f"psy0_{mc}_{tb}")
                    ps_y1 = psf.tile([128, 512], F32, tag="yps1", bufs=2,
                                     name=f"psy1_{mc}_{tb}")
                    for f in range(FT):
                        hbl = hb_tiles[f][:, tloc:tloc + 128]
                        nc.tensor.matmul(ps_y0, hbl, w2_tiles[f][:, 0:512],
                                         start=(f == 0), stop=(f == FT - 1))
                        nc.tensor.matmul(ps_y1, hbl, w2_tiles[f][:, 512:DM],
                                         start=(f == 0), stop=(f == FT - 1))

                    ps_xt = psf.tile([128, DM], BF16, tag="xtr", bufs=1,
                                     name=f"xtr{mc}_{tb}")
                    for co in range(CT):
                        nc.tensor.transpose(
                            ps_xt[:, ts(co, 128)],
                            x1_tiles[co][:, t0:t0 + 128], ident_b)
                    x1t = sf.tile([128, DM], BF16, tag="x1t", bufs=2,
                                  name=f"x1t{mc}_{tb}")
                    nc.vector.scalar_tensor_tensor(
                        x1t, ps_xt, 1.0, be2bc, op0=AL.mult, op1=AL.add)

                    yt = sf.tile([128, DM], BF16, tag="yt", bufs=2,
                                 name=f"yt{mc}_{tb}")
                    s_a = sf.tile([128, 1], F32, tag="sa", bufs=2,
                                  name=f"sa{mc}_{tb}")
                    s_b = sf.tile([128, 1], F32, tag="sb", bufs=2,
                                  name=f"sb{mc}_{tb}")
                    nc.vector.scalar_tensor_tensor(
                        yt[:, 0:512], ps_y0, 1.0, b2bc[:, 0:512],
                        op0=AL.mult, op1=AL.add, accum_out=s_a)
                    nc.vector.scalar_tensor_tensor(
                        yt[:, 512:DM], ps_y1, 1.0, b2bc[:, 512:DM],
                        op0=AL.mult, op1=AL.add, accum_out=s_b)
                    s_t = sf.tile([128, 1], F32, tag="st", bufs=2,
                                  name=f"st{mc}_{tb}")
                    nc.vector.tensor_add(s_t, s_a, s_b)
                    sq = sf.tile([128, DM], BF16, tag="sq", bufs=2,
                                 name=f"sq{mc}_{tb}")
                    ssq = sf.tile([128, 1], F32, tag="ssq", bufs=2,
                                  name=f"ssq{mc}_{tb}")
                    nc.vector.scalar_tensor_tensor(sq, yt, 1.0, yt,
                                                   op0=AL.mult, op1=AL.mult,
                                                   accum_out=ssq)
                    mean = sf.tile([128, 1], F32, tag="mean", bufs=2,
                                   name=f"mean{mc}_{tb}")
                    nc.vector.tensor_scalar_mul(mean, s_t, 1.0 / DM)
                    e2f = sf.tile([128, 1], F32, tag="e2f", bufs=2,
                                  name=f"e2f{mc}_{tb}")
                    nc.vector.tensor_scalar_mul(e2f, ssq, 1.0 / DM)
                    m2f = sf.tile([128, 1], F32, tag="m2f", bufs=2,
                                  name=f"m2f{mc}_{tb}")
                    nc.vector.tensor_mul(m2f, mean, mean)
                    varf = sf.tile([128, 1], F32, tag="varf", bufs=2,
                                   name=f"varf{mc}_{tb}")
                    nc.vector.tensor_sub(varf, e2f, m2f)
                    sdf = sf.tile([128, 1], F32, tag="sdf", bufs=2,
                                  name=f"sdf{mc}_{tb}")
                    nc.scalar.activation(sdf, varf, AF.Sqrt, bias=eps_c)
                    rstd = sf.tile([128, 1], F32, tag="rstd", bufs=2,
                                   name=f"rstd{mc}_{tb}")
                    nc.vector.reciprocal(rstd, sdf)

                    vn = sf.tile([128, DM], BF16, tag="vn", bufs=2,
                                 name=f"vn{mc}_{tb}")
                    nc.vector.tensor_scalar(vn, yt, mean, rstd,
                                            op0=AL.subtract, op1=AL.mult)
                    t1 = sf.tile([128, DM], BF16, tag="sq", bufs=2,
                                 name=f"t1{mc}_{tb}")
                    nc.vector.tensor_mul(t1, vn, g2bc)
                    yout = sf.tile([128, DM], F32, tag="yout", bufs=2,
                                   name=f"yout{mc}_{tb}")
                    nc.vector.tensor_add(yout, t1, x1t)
                    # positions t0..t0+127 are tokens 4l+r with r = t0//512,
                    # l = (t0 % 512) + 0..127
                    rr, l0 = t0 // (T // 4), t0 % (T // 4)
                    nc.sync.dma_start(
                        out=y_ap.rearrange("(l k) c -> k l c", k=KER)[
                            rr, l0:l0 + 128, :],
                        in_=yout)

        x1_ctx.close()

    nc.compile()
    return nc


_PROGRAM_CACHE = {}


def _get_program(S=S_FULL):
    if S not in _PROGRAM_CACHE:
        _PROGRAM_CACHE[S] = build_program(S)
    return _PROGRAM_CACHE[S]


def _vec_fold(v, cols):
    """[N] -> [128, N//128] with column i = v[i*128:(i+1)*128]."""
    v = np.asarray(v, np.float32)
    return np.ascontiguousarray(v.reshape(cols, 128).T)


def prep_inputs(inputs, S=S_FULL):
    import ml_dtypes
    BD = ml_dtypes.bfloat16
    T = S // 2
    L = S // KER
    LLOC = L // 2

    g = {k: np.asarray(v, np.float32) for k, v in inputs.items()}

    def wtile(w, nt):
        ci = w.shape[0] // 128
        return np.ascontiguousarray(
            w.reshape(ci, 128, nt, 128).transpose(2, 1, 0, 3)
            .reshape(nt, 128, ci * 128).astype(BD))

    w2t = np.ascontiguousarray(g["w2"].reshape(FT, 128, DM).astype(BD))

    shared = {
        "wq": wtile(g["wq"] * NORM, CT), "wk": wtile(g["wk"] * NORM, CT),
        "wv": wtile(g["wv"], CT), "wc": wtile(g["wc"], CT),
        "w1": wtile(g["w1"], FT), "w2": w2t,
        "wup": g["wup"].astype(BD),
        "g2bc": np.ascontiguousarray(
            np.tile(g["g2"].reshape(1, DM), (128, 1)).astype(BD)),
        "be2bc": np.ascontiguousarray(
            np.tile(g["be2"].reshape(1, DM), (128, 1)).astype(BD)),
        "b2bc": np.ascontiguousarray(
            np.tile(g["b2"].reshape(1, DM), (128, 1)).astype(BD)),
    }
    cvec = np.zeros((128, NCONST), np.float32)

    def setc(nm, arr):
        off, w = _COFF[nm]
        assert arr.shape == (128, w), (nm, arr.shape)
        cvec[:, off:off + w] = arr

    setc("bqn", _vec_fold(g["bq"] * NORM, CT))
    setc("bkn", _vec_fold(g["bk"] * NORM, CT))
    for nm in ("bv", "bc", "g1", "be1", "b2"):
        src = {"bv": "bv", "bc": "bc", "g1": "g1", "be1": "be1",
               "b2": "b2"}[nm]
        setc(nm, _vec_fold(g[src], CT))
    for nm in ("dbq", "dbk", "dbv"):
        setc(nm, _vec_fold(g[nm], CT))
    setc("b1", _vec_fold(g["b1"], FT))
    setc("bup", np.tile(g["bup"].reshape(DD), 2).reshape(128, 1))
    # combined FIR taps g_j[c] = (1/KER) * sum_{i+m=j, 0<=i<KW, 0<=m<KER} k_i
    for nm, knm in (("firq", "dwq"), ("firk", "dwk"), ("firv", "dwv")):
        kk = g[knm]  # [KW, DM]
        taps = np.zeros((NTAP, DM), np.float32)
        for jj in range(NTAP):
            for i in range(KW):
                m = jj - i
                if 0 <= m < KER:
                    taps[jj] += kk[i]
        taps *= 1.0 / KER
        # [NTAP, DM] -> [128, CT*NTAP] with cols co*NTAP+j
        setc(nm, taps.T.reshape(CT, 128, NTAP).transpose(1, 0, 2)
             .reshape(128, CT * NTAP))

    TH = T + HALO
    QB = TH // 4
    in_maps = []
    for c in range(N_CORES):
        b, hf = c // 2, c % 2
        m = dict(shared)
        fm = (g["x_enc"][b] + g["x_pos"][b]).T  # [DM, S]
        if hf == 0:
            sl = np.concatenate(
                [np.zeros((DM, HALO), np.float32), fm[:, :T]], axis=1)
        else:
            sl = fm[:, T - HALO:2 * T]
        # residue-major: col r*QB+q = token col 4q+r
        m["xemb"] = np.ascontiguousarray(
            sl.reshape(DM, QB, 4).transpose(0, 2, 1).reshape(DM, TH)
            .astype(BD))
        cv = cvec.copy()
        cv[:, _COFF["hmask"][0]:_COFF["hmask"][0] + HALO] = float(hf)
        for nm, dbn in (("fxq", "dbq"), ("fxk", "dbk"), ("fxv", "dbv")):
            cv[:, _COFF[nm][0]:_COFF[nm][0] + CT] = \
                -0.75 * (1.0 - hf) * _vec_fold(g[dbn], CT)
        m["cvec"] = cv
        m["mask"] = np.ascontiguousarray(
            (np.arange(L)[:, None] <= (hf * LLOC + np.arange(LLOC))[None, :])
            .astype(BD))
        in_maps.append(m)
    return in_maps


def gather_output(results, S=S_FULL):
    T = S // 2
    y = np.empty((B, S, DM), np.float32)
    for c in range(N_CORES):
        b, hf = c // 2, c % 2
        y[b, hf * T:(hf + 1) * T, :] = results[c]["y"]
    return y


def kernel(**inputs):
    nc = _get_program(S_FULL)
    in_maps = prep_inputs(inputs, S_FULL)
    res = run_bass_kernel_spmd(nc, in_maps, list(range(N_CORES)))
    return gather_output(res.results, S_FULL)
